# revision 1
# baseline (speedup 1.0000x reference)
"""DBSCAN (eps=22, min_samples=5) on X[8192, 256] float32, distributed
across 8 TRN2 NeuronCores via Bass/Tile.

Math (mirrors the jax reference):
  d2[i,j] = ||x_i||^2 + ||x_j||^2 - 2 (X X^T)[i,j]
  adj     = d2 <= eps^2
  core_i  = rowsum(adj) >= min_samples
  comp    = min-index label propagation over the core-core eps-graph
  labels  = component ids in scan order; border points attach to the
            min-index core neighbor; rest are noise (-1).

Sharding: core m owns rows S_m = [1024*m, 1024*(m+1)).  Each core:
  phase A: computes its [1024, 8192] Gram block on the tensor engine
           (bf16 inputs, fp32 PSUM; a K=2 bf16 hi/lo rank-2 update folds
           the column threshold in), evicts to a bf16 0/1 adjacency via
           tensor_scalar(is_ge, per-partition row threshold) on DVE, and
           accumulates row degrees on the scalar engine (activation
           accum_out = free-dim sum) — three engines in parallel.
  phase C: min-label propagation in negated encoding n = BIG - comp
           (so "BIG" = 0 and non-edges/non-core contribute the max-identity
           0): per iteration, PE broadcasts the gathered n-vector into
           PSUM via a ones outer product, the scalar engine evicts a copy
           to SBUF, and each 128-row chunk computes
           max_j adj[i,j] * n_j via tensor_tensor(mult) + tensor_reduce(max)
           — the multiplies split between the vector engine (reading PSUM)
           and gpsimd (reading the SBUF copy) to share the streaming load.
           A 4 KB AllGather shares the updated own-chunk between iterations.

Propagation runs a fixed 3 iterations; the host verifies the fixpoint
(iter2 == iter3 — the exact while-loop exit condition of the reference)
and falls back to a full numpy recomputation if it has not converged
(it has: this dataset converges after 2 iterations).  The tiny O(N)
label-numbering tail (cumsum over representatives, border attachment)
runs on the host.
"""

import numpy as np
import ml_dtypes

N = 8192
D = 256
NCORES = 8
NPC = N // NCORES          # 1024 rows per core
RCH = NPC // 128           # 8 row-chunks of 128 per core
EPS2 = 484.0               # 22.0**2
MIN_SAMPLES = 5
BIG = N
NITER = 3                  # fixpoint after 2 on this data; 3rd proves it

_CACHE = {}


def _get_maxred_op():
    """Register (once) a custom DVE op:
        out      = in0 * in1
        accum_out = max(s0, max_k out[:, k])
    i.e. the TENSOR_TENSOR_REDUCE production op with a MAX fold."""
    if "maxred" in _CACHE:
        return _CACHE["maxred"]
    from concourse import dve_ops as dv
    from concourse.dve_spec import Spec, Src0, Src1, C0, maxx, lower
    from concourse.dve_uop import DveOpSpec

    name = "TT_MAXRED_DBSCAN_ANT"
    existing = [op for op in dv.OPS if op.name == name]
    if existing:
        _CACHE["maxred"] = existing[0]
        return existing[0]

    def _ref(in0, in1, c0, c1, c2):
        b = (np.asarray(in0, np.float32) * np.asarray(in1, np.float32)).astype(
            np.float32)
        red = b.reshape(b.shape[0], -1).max(axis=-1, keepdims=True)
        return b, np.maximum(np.asarray(c0, np.float32), red)

    spec = Spec(body=Src0 * Src1, accum=maxx, accum_init=C0, reference=_ref)
    op = dv.DveOp(name, spec, subdim=False, uops_sha={})
    dv.OPS.append(op)
    dv.CUSTOM_DVE_SPECS[name] = spec
    dv._SUB_OPCODE_FOR_NAME[name] = dv._CUSTOM_DVE_ROW_BASE + len(dv.OPS) - 1
    assert dv._SUB_OPCODE_FOR_NAME[name] < 0x20
    # pin the uops sha so DveOp.compile()'s drift check passes
    for ver in ("v3", "v4"):
        try:
            s = DveOpSpec(
                name=name,
                opcode=dv.get_dve_sub_opcode(name),
                uops=lower(spec, ver=ver),
                rd1_en=dv.has_src1(spec),
            )
            op.uops_sha[ver] = s.sha(ver)
        except Exception:
            pass
    _CACHE["maxred"] = op
    return op


def _build_nc():
    import concourse.bass as bass
    import concourse.bacc as bacc
    import concourse.tile as tile
    import concourse.mybir as mybir

    f32 = mybir.dt.float32
    bf16 = mybir.dt.bfloat16
    Alu = mybir.AluOpType
    Act = mybir.ActivationFunctionType

    nc = bacc.Bacc("TRN2", target_bir_lowering=False, debug=False,
                   num_devices=NCORES)

    # ---- kernel I/O ----
    xt_d = nc.declare_dram_parameter("xt", [2, 128, N], bf16, isOutput=False)
    xo_d = nc.declare_dram_parameter("xtown", [2, 128, NPC], bf16, isOutput=False)
    cn_d = nc.declare_dram_parameter("cneg", [2, N], bf16, isOutput=False)
    rj_d = nc.declare_dram_parameter("rj", [128, RCH], f32, isOutput=False)
    ix_d = nc.declare_dram_parameter("idxn", [128, RCH], f32, isOutput=False)

    deg_o = nc.declare_dram_parameter("deg", [128, RCH], f32, isOutput=True)
    bord_o = nc.declare_dram_parameter("bord", [128, RCH], f32, isOutput=True)
    c2_o = nc.declare_dram_parameter("comp2", [128, RCH], f32, isOutput=True)
    c3_o = nc.declare_dram_parameter("comp3", [128, RCH], f32, isOutput=True)

    with tile.TileContext(nc) as tc:
        with (
            tc.tile_pool(name="adj", bufs=1) as adjp,
            tc.tile_pool(name="misc", bufs=1) as misc,
            tc.tile_pool(name="dram", bufs=1, space="DRAM") as dram,
        ):
            adj = [adjp.tile([128, N], bf16, tag=f"adj{r}", name=f"adj{r}")
                   for r in range(RCH)]

            rj = misc.tile([128, RCH], f32, tag="rj")
            nc.sync.dma_start(rj[:], rj_d[:])
            idxn = misc.tile([128, RCH], f32, tag="idxn")
            nc.sync.dma_start(idxn[:], ix_d[:])

            deg_sb = misc.tile([128, RCH], f32, tag="deg")
            core01 = misc.tile([128, RCH], f32, tag="core01")
            comp0 = misc.tile([128, RCH], f32, tag="comp0")
            ones1 = misc.tile([1, 128], f32, tag="ones1")
            nc.gpsimd.memset(ones1[:], 1.0)

            # ---------------- phase A: G block + adjacency + degree ------
            with (
                tc.tile_pool(name="xt", bufs=1) as xtp,
                tc.tile_pool(name="psA", bufs=4, space=bass.MemorySpace.PSUM) as psA,
            ):
                xt0 = xtp.tile([128, N], bf16, tag="xt0")
                nc.sync.dma_start(xt0[:], xt_d[0])
                xt1 = xtp.tile([128, N], bf16, tag="xt1")
                nc.sync.dma_start(xt1[:], xt_d[1])
                xo0 = xtp.tile([128, NPC], bf16, tag="xo0")
                nc.sync.dma_start(xo0[:], xo_d[0])
                xo1 = xtp.tile([128, NPC], bf16, tag="xo1")
                nc.sync.dma_start(xo1[:], xo_d[1])
                cn = xtp.tile([2, N], bf16, tag="cn")
                nc.sync.dma_start(cn[:], cn_d[:])
                ones2 = xtp.tile([2, 128], bf16, tag="ones2")
                nc.gpsimd.memset(ones2[:], 1.0)
                junk = xtp.tile([128, N], bf16, tag="junk")

                for r in range(RCH):
                    # own-row lhsT: local column c = p*8 + r  ->  [:, r::8]
                    l0 = xo0[:, r::RCH]
                    l1 = xo1[:, r::RCH]
                    for n in range(16):
                        g = psA.tile([128, 512], f32, tag="g", name="g")
                        sl = slice(n * 512, (n + 1) * 512)
                        nc.tensor.matmul(g[:], l0, xt0[:, sl], start=True, stop=False)
                        nc.tensor.matmul(g[:], l1, xt1[:, sl], start=False, stop=False)
                        # rank-2 bf16 hi/lo update adds -(sq_j/2 - eps2/4)
                        nc.tensor.matmul(g[:], ones2[:], cn[:, sl],
                                         start=False, stop=True)
                        # adj = (g >= rj_i)
                        nc.vector.tensor_scalar(
                            out=adj[r][:, sl], in0=g[:],
                            scalar1=rj[:, r:r + 1], scalar2=None, op0=Alu.is_ge)
                    # degree: free-dim sum on the scalar engine
                    nc.scalar.activation(
                        junk[:], adj[r][:], Act.Identity,
                        bias=0.0, scale=1.0,
                        accum_out=deg_sb[:, r:r + 1])

            # ---------------- core mask + n_0 = core * (BIG - idx) -------
            nc.vector.tensor_scalar(
                out=core01[:], in0=deg_sb[:], scalar1=float(MIN_SAMPLES),
                scalar2=None, op0=Alu.is_ge,
            )
            nc.vector.tensor_mul(comp0[:], core01[:], idxn[:])

            # ---------------- phase C: label propagation (negated) -------
            with (
                tc.tile_pool(name="psB", bufs=1, space=bass.MemorySpace.PSUM) as psB,
                tc.tile_pool(name="scr", bufs=1) as scrp,
            ):
                scr = scrp.tile([128, 4096], f32, tag="scr")
                scr2 = scrp.tile([128, 4096], f32, tag="scr2")
                nb = scrp.tile([128, 4096], f32, tag="nb")
                comp_cur = comp0
                acc1 = None
                comp_iters = []
                for t in range(1, NITER + 1):
                    # all-gather own n chunk -> full n vector
                    agi = dram.tile([128, RCH], f32, tag=f"agi{t}", name=f"agi{t}")
                    nc.gpsimd.dma_start(agi[:], comp_cur[:])
                    ago = dram.tile([1, N], f32, tag=f"ago{t}", name=f"ago{t}",
                                    addr_space="Shared")
                    nc.gpsimd.collective_compute(
                        "AllGather",
                        Alu.bypass,
                        replica_groups=[list(range(NCORES))],
                        ins=[agi[:].opt()],
                        outs=[ago[:].opt()],
                    )

                    acc = misc.tile([128, RCH], f32, tag=f"acc{t}", name=f"acc{t}")
                    mh = misc.tile([128, 2 * RCH], f32, tag=f"mh{t}", name=f"mh{t}")
                    NGP = 5   # row-chunks whose product runs on GpSimd
                    for h in range(2):
                        crowh = scrp.tile([1, 4096], f32, tag="crowh",
                                          name=f"crowh{t}{h}")
                        nc.gpsimd.dma_start(
                            crowh[:], ago[0:1, h * 4096:(h + 1) * 4096])
                        pb = psB.tile([128, 4096], f32, tag="pb", name="pb")
                        for q in range(8):
                            sl = slice(q * 512, (q + 1) * 512)
                            nc.tensor.matmul(
                                pb[:, sl], ones1[:],
                                crowh[0:1, q * 512:(q + 1) * 512],
                                start=True, stop=True,
                            )
                        nc.scalar.copy(nb[:], pb[:])
                        for r in range(RCH):
                            if r < NGP:
                                nc.gpsimd.tensor_tensor(
                                    out=scr2[:],
                                    in0=adj[r][:, h * 4096:(h + 1) * 4096],
                                    in1=nb[:],
                                    op=Alu.mult,
                                )
                                src_t = scr2
                            else:
                                nc.vector.tensor_tensor(
                                    out=scr[:],
                                    in0=adj[r][:, h * 4096:(h + 1) * 4096],
                                    in1=pb[:],
                                    op=Alu.mult,
                                )
                                src_t = scr
                            nc.vector.tensor_reduce(
                                out=mh[:, 2 * r + h:2 * r + h + 1],
                                in_=src_t[:],
                                axis=mybir.AxisListType.X,
                                op=Alu.max,
                            )
                    for r in range(RCH):
                        nc.vector.tensor_tensor(
                            out=acc[:, r:r + 1],
                            in0=mh[:, 2 * r:2 * r + 1],
                            in1=mh[:, 2 * r + 1:2 * r + 2],
                            op=Alu.max,
                        )
                    if t == 1:
                        acc1 = acc
                    compn = misc.tile([128, RCH], f32, tag=f"comp{t}",
                                      name=f"comp{t}")
                    nc.vector.tensor_mul(compn[:], core01[:], acc[:])
                    comp_iters.append(compn)
                    comp_cur = compn

            # ---------------- outputs ------------------------------------
            nc.sync.dma_start(deg_o[:], deg_sb[:])
            nc.sync.dma_start(bord_o[:], acc1[:])
            nc.sync.dma_start(c2_o[:], comp_iters[1][:])
            nc.sync.dma_start(c3_o[:], comp_iters[2][:])

    nc.compile()
    return nc


def _prepare_inputs(X):
    X = np.ascontiguousarray(X, dtype=np.float32)
    sq = np.sum(X * X, axis=1, dtype=np.float32)          # [N]
    # adj  <=>  G >= (sq_i/2 - eps2/4) + (sq_j/2 - eps2/4)
    thr = sq * np.float32(0.5) - np.float32(EPS2 / 4.0)   # [N]

    xt_bf = X.T.astype(ml_dtypes.bfloat16)                # [256, 8192]
    xt = np.ascontiguousarray(xt_bf.reshape(2, 128, N))

    cneg_f = (-thr).astype(np.float32)
    hi = cneg_f.astype(ml_dtypes.bfloat16)
    lo = (cneg_f - hi.astype(np.float32)).astype(ml_dtypes.bfloat16)
    cneg = np.ascontiguousarray(np.stack([hi, lo], axis=0))  # [2, 8192]

    idx = np.arange(N, dtype=np.float32)
    in_maps = []
    for m in range(NCORES):
        rows = np.arange(m * NPC, (m + 1) * NPC)
        # local i = p*RCH + r  ->  [128, RCH] layout
        rows_pr = rows.reshape(128, RCH)
        in_maps.append({
            "xt": xt,
            "cneg": cneg,
            "xtown": np.ascontiguousarray(xt_bf[:, rows].reshape(2, 128, NPC)),
            "rj": np.ascontiguousarray(thr[rows_pr]),
            # negated index encoding: n_0 = BIG - idx (for core points)
            "idxn": np.ascontiguousarray((BIG - idx)[rows_pr]),
        })
    return in_maps


def _host_finish(deg, bord, comp):
    """Exact numpy port of the reference's label-numbering tail."""
    idx = np.arange(N, dtype=np.int64)
    core = deg >= MIN_SAMPLES
    is_rep = core & (comp == idx)
    cid = np.cumsum(is_rep.astype(np.int64)) - 1
    comp_safe = np.minimum(comp, N - 1)
    core_label = np.where(core, cid[comp_safe], -1)
    first_core_nb = bord
    has_nb = first_core_nb < N
    nb_safe = np.minimum(first_core_nb, N - 1)
    border_label = np.where(has_nb, core_label[nb_safe], -1)
    return np.where(core, core_label, border_label).astype(np.int64)


def _host_fallback(X):
    """Full-precision numpy recomputation (only used if the device
    propagation has not reached the fixpoint, which does not happen)."""
    X = np.asarray(X, dtype=np.float32)
    sq = np.sum(X * X, axis=1, dtype=np.float32)
    G = X @ X.T
    d2 = sq[:, None] + sq[None, :] - 2.0 * G
    adj = d2 <= np.float32(EPS2)
    deg = adj.sum(1)
    core = deg >= MIN_SAMPLES
    idx = np.arange(N, dtype=np.int64)
    comp = np.where(core, idx, BIG)
    adjc = adj & core[None, :]
    while True:
        new = comp.copy()
        for s in range(0, N, 1024):
            cand = np.where(adjc[s:s + 1024], comp[None, :], BIG).min(1)
            new[s:s + 1024] = np.minimum(comp[s:s + 1024], cand)
        new = np.where(core, new, BIG)
        if (new == comp).all():
            break
        comp = new
    bord = np.where(adjc, idx[None, :], BIG).min(1)
    return _host_finish(deg.astype(np.int64), bord, comp)


def _flatten_out(arrs):
    """[8 cores][128, RCH] -> [8192] in global row order."""
    return np.concatenate([np.asarray(a, np.float32).reshape(-1) for a in arrs])


def _run_device(in_maps):
    from concourse import bass_utils
    if "nc" not in _CACHE:
        _CACHE["nc"] = _build_nc()
    res = bass_utils.run_bass_kernel_spmd(
        _CACHE["nc"], in_maps, list(range(NCORES)))
    return res.results


def kernel(X):
    in_maps = _prepare_inputs(X)
    results = _run_device(in_maps)

    deg = _flatten_out([results[m]["deg"] for m in range(NCORES)])
    nbord = _flatten_out([results[m]["bord"] for m in range(NCORES)])
    n2 = _flatten_out([results[m]["comp2"] for m in range(NCORES)])
    n3 = _flatten_out([results[m]["comp3"] for m in range(NCORES)])

    if not np.array_equal(n2, n3):
        return _host_fallback(X)

    # decode the negated encoding: comp = BIG - n  (n = 0 -> BIG sentinel)
    comp = BIG - np.rint(n3).astype(np.int64)
    bord = BIG - np.rint(nbord).astype(np.int64)
    degi = np.rint(deg).astype(np.int64)
    return _host_finish(degi, bord, comp)



# revision 5
# speedup vs baseline: 6.8132x; 6.8132x over previous
"""DBSCAN (eps=22, min_samples=5) on X[8192, 256] float32, distributed
across 8 TRN2 NeuronCores via Bass/Tile.

Math (mirrors the jax reference):
  d2[i,j] = ||x_i||^2 + ||x_j||^2 - 2 (X X^T)[i,j]
  adj     = d2 <= eps^2   <=>   G[i,j] >= thr_i + thr_j,
            thr = ||x||^2/2 - eps^2/4
  core_i  = rowsum(adj) >= min_samples
  comp    = min-index connected components of the core-core eps-graph
  labels  = component ids in scan order; border points attach to the
            min-index core neighbor; rest are noise (-1).

Device (the compute-bound part): core m owns rows [1024*m, 1024*(m+1)).
Each core computes its [1024, 8192] Gram block on the tensor engine
(bf16 inputs, fp32 PSUM) and immediately binarizes each [128, 512] PSUM
tile into a 0/1 uint8 adjacency byte:
  - 12/16 column tiles: DVE scalar_tensor_tensor
        adj = (g - thr_i) is_ge thr_j    (thr_j from an f32 broadcast tile)
  - 4/16 column tiles: a K=2 bf16 hi/lo matmul folds -thr_j into PSUM,
    then the scalar engine evicts with Sign(g~ - thr_i) (bias per
    partition); decode on host is byte == 1 either way.
The 8 MB/core of adjacency bytes stream to DRAM while the PE computes.
No collectives; the PE Gram is the critical path.

Host: degrees (popcount), connected components of the core-core graph
(packed-bit BFS in increasing index order, so each component's label is
its min core index — exactly the reference's propagation fixpoint),
border attachment, and scan-order cluster numbering. All exact integer
math on the device-computed adjacency.
"""

import numpy as np
import ml_dtypes

N = 8192
D = 256
NCORES = 8
NPC = N // NCORES          # 1024 rows per core
RCH = NPC // 128           # 8 row-chunks of 128 per core
CT = N // 512              # 16 column tiles of 512
EPS2 = 484.0               # 22.0**2
MIN_SAMPLES = 5
BIG = N

# column tiles evicted via the scalar engine (Sign); rest go to DVE
ACT_TILES = frozenset((3, 7, 11, 15))

_CACHE = {}


def _build_nc():
    import concourse.bass as bass
    import concourse.bacc as bacc
    import concourse.tile as tile
    import concourse.mybir as mybir

    f32 = mybir.dt.float32
    bf16 = mybir.dt.bfloat16
    u8 = mybir.dt.uint8
    Alu = mybir.AluOpType
    Act = mybir.ActivationFunctionType

    nc = bacc.Bacc("TRN2", target_bir_lowering=False, debug=False,
                   num_devices=NCORES)

    # ---- kernel I/O ----
    xt_d = nc.declare_dram_parameter("xt", [2, 128, N], bf16, isOutput=False)
    xo_d = nc.declare_dram_parameter("xtown", [2, 128, NPC], bf16,
                                     isOutput=False)
    cth_d = nc.declare_dram_parameter("cth", [128, N], f32, isOutput=False)
    cn_d = nc.declare_dram_parameter("cneg", [2, N], bf16, isOutput=False)
    rj_d = nc.declare_dram_parameter("rj", [128, RCH], f32, isOutput=False)
    rjn_d = nc.declare_dram_parameter("rjn", [128, RCH], f32, isOutput=False)

    adj_o = nc.declare_dram_parameter("adj", [RCH, 2, 128, N // 2], u8,
                                      isOutput=True)

    NG = 8                 # xt / cth column groups (1024 cols each)
    GW = N // NG

    with tile.TileContext(nc) as tc:
        with (
            tc.tile_pool(name="inp", bufs=1) as inp,
            tc.tile_pool(name="adjp", bufs=1) as adjp,
            tc.tile_pool(name="ps", bufs=8, space=bass.MemorySpace.PSUM) as ps,
        ):
            # small inputs first so the first matmuls can start early
            xo0 = inp.tile([128, NPC], bf16, tag="xo0")
            nc.sync.dma_start(xo0[:], xo_d[0])
            xo1 = inp.tile([128, NPC], bf16, tag="xo1")
            nc.sync.dma_start(xo1[:], xo_d[1])
            rj = inp.tile([128, RCH], f32, tag="rj")
            nc.sync.dma_start(rj[:], rj_d[:])
            rjn = inp.tile([128, RCH], f32, tag="rjn")
            nc.sync.dma_start(rjn[:], rjn_d[:])
            cn = inp.tile([2, N], bf16, tag="cn")
            nc.sync.dma_start(cn[:], cn_d[:])
            ones2 = inp.tile([2, 128], bf16, tag="ones2")
            nc.gpsimd.memset(ones2[:], 1.0)

            # streamed inputs, grouped so compute overlaps the loads
            xtg = [[None] * NG for _ in range(2)]
            cthg = [None] * NG
            for g in range(NG):
                sl = slice(g * GW, (g + 1) * GW)
                t0 = inp.tile([128, GW], bf16, tag=f"xt0g{g}")
                nc.sync.dma_start(t0[:], xt_d[0][:, sl])
                xtg[0][g] = t0
                t1 = inp.tile([128, GW], bf16, tag=f"xt1g{g}")
                nc.gpsimd.dma_start(t1[:], xt_d[1][:, sl])
                xtg[1][g] = t1
                tc_ = inp.tile([128, GW], f32, tag=f"cthg{g}")
                nc.scalar.dma_start(tc_[:], cth_d[:, sl])
                cthg[g] = tc_

            adjt = [[adjp.tile([128, N // 2], u8, tag=f"adj{r}h{h}",
                               name=f"adj{r}h{h}")
                     for h in range(2)] for r in range(RCH)]

            for r in range(RCH):
                l0 = xo0[:, r * 128:(r + 1) * 128]
                l1 = xo1[:, r * 128:(r + 1) * 128]
                for n in range(CT):
                    gidx, goff = divmod(n, 2)
                    csl = slice(goff * 512, goff * 512 + 512)
                    h, hoff = divmod(n, 8)
                    osl = slice(hoff * 512, hoff * 512 + 512)
                    g = ps.tile([128, 512], f32, tag="g", name="g")
                    if n in ACT_TILES:
                        nc.tensor.matmul(g[:], l0, xtg[0][gidx][:, csl],
                                         start=True, stop=False)
                        nc.tensor.matmul(g[:], l1, xtg[1][gidx][:, csl],
                                         start=False, stop=False)
                        # K=2 bf16 hi/lo rank-2 update adds -thr_j
                        nc.tensor.matmul(g[:], ones2[:],
                                         cn[:, n * 512:(n + 1) * 512],
                                         start=False, stop=True)
                        # adj = Sign(g~ - thr_i): +1 -> byte 1
                        nc.scalar.activation(
                            adjt[r][h][:, osl], g[:], Act.Sign,
                            bias=rjn[:, r:r + 1], scale=1.0)
                    else:
                        nc.tensor.matmul(g[:], l0, xtg[0][gidx][:, csl],
                                         start=True, stop=False)
                        nc.tensor.matmul(g[:], l1, xtg[1][gidx][:, csl],
                                         start=False, stop=True)
                        # adj = (g - thr_i) >= thr_j
                        nc.vector.scalar_tensor_tensor(
                            out=adjt[r][h][:, osl], in0=g[:],
                            scalar=rj[:, r:r + 1],
                            in1=cthg[gidx][:, csl],
                            op0=Alu.subtract, op1=Alu.is_ge)
                    if n == 7:
                        nc.gpsimd.dma_start(adj_o[r][0], adjt[r][0][:])
                    elif n == 15:
                        nc.gpsimd.dma_start(adj_o[r][1], adjt[r][1][:])

    nc.compile()
    return nc


def _prepare_inputs(X):
    X = np.ascontiguousarray(X, dtype=np.float32)
    sq = np.sum(X * X, axis=1, dtype=np.float32)          # [N]
    # adj  <=>  G >= thr_i + thr_j
    thr = sq * np.float32(0.5) - np.float32(EPS2 / 4.0)   # [N]

    xt_bf = X.T.astype(ml_dtypes.bfloat16)                # [256, 8192]
    xt = np.ascontiguousarray(xt_bf.reshape(2, 128, N))

    cth = np.ascontiguousarray(np.broadcast_to(thr, (128, N)))

    cneg_f = (-thr).astype(np.float32)
    hi = cneg_f.astype(ml_dtypes.bfloat16)
    lo = (cneg_f - hi.astype(np.float32)).astype(ml_dtypes.bfloat16)
    cneg = np.ascontiguousarray(np.stack([hi, lo], axis=0))  # [2, 8192]

    in_maps = []
    for m in range(NCORES):
        rows = np.arange(m * NPC, (m + 1) * NPC)
        # chunk r holds local rows r*128 .. r*128+127 -> rj[q, r]
        rj = np.ascontiguousarray(thr[rows].reshape(RCH, 128).T)
        in_maps.append({
            "xt": xt,
            "xtown": np.ascontiguousarray(xt_bf[:, rows].reshape(2, 128, NPC)),
            "cth": cth,
            "cneg": cneg,
            "rj": rj,
            "rjn": np.ascontiguousarray(-rj),
        })
    return in_maps


def _host_finish(deg, bord, comp):
    """Exact numpy port of the reference's label-numbering tail."""
    idx = np.arange(N, dtype=np.int64)
    core = deg >= MIN_SAMPLES
    is_rep = core & (comp == idx)
    cid = np.cumsum(is_rep.astype(np.int64)) - 1
    comp_safe = np.minimum(comp, N - 1)
    core_label = np.where(core, cid[comp_safe], -1)
    first_core_nb = bord
    has_nb = first_core_nb < N
    nb_safe = np.minimum(first_core_nb, N - 1)
    border_label = np.where(has_nb, core_label[nb_safe], -1)
    return np.where(core, core_label, border_label).astype(np.int64)


def _host_labels(A):
    """Exact DBSCAN labeling from the 0/1 uint8 adjacency matrix."""
    deg = A.sum(axis=1, dtype=np.int64)
    core = deg >= MIN_SAMPLES
    idx = np.arange(N, dtype=np.int64)

    # core-core adjacency as packed bits for fast BFS
    core_u8 = core.astype(np.uint8)
    bits = np.packbits(A * core_u8[None, :], axis=1)      # [N, N//8]

    comp = np.full(N, BIG, dtype=np.int64)
    visited = np.zeros(N, dtype=bool)
    for i in np.nonzero(core)[0]:
        if visited[i]:
            continue
        member = np.zeros(N // 8, dtype=np.uint8)
        member[i >> 3] = 0x80 >> (i & 7)
        frontier = np.array([i], dtype=np.int64)
        while frontier.size:
            reach = np.bitwise_or.reduce(bits[frontier], axis=0)
            new = reach & ~member
            member |= new
            frontier = np.nonzero(np.unpackbits(new))[0]
        rows = np.nonzero(np.unpackbits(member))[0]
        comp[rows] = i          # scan order => i is the min index
        visited[rows] = True

    # border points: min-index core neighbor
    bord = np.full(N, BIG, dtype=np.int64)
    nonc = np.nonzero(~core)[0]
    if nonc.size:
        ub = np.unpackbits(bits[nonc], axis=1)[:, :N]
        has = ub.any(axis=1)
        bord[nonc[has]] = ub[has].argmax(axis=1)

    return _host_finish(deg, bord, comp)


def _run_device(in_maps):
    from concourse import bass_utils
    if "nc" not in _CACHE:
        _CACHE["nc"] = _build_nc()
    res = bass_utils.run_bass_kernel_spmd(
        _CACHE["nc"], in_maps, list(range(NCORES)))
    return res.results


def kernel(X):
    in_maps = _prepare_inputs(X)
    results = _run_device(in_maps)

    A = np.empty((N, N), dtype=np.uint8)
    for m in range(NCORES):
        blk = np.asarray(results[m]["adj"])       # [RCH, 2, 128, N//2]
        blk = blk.transpose(0, 2, 1, 3).reshape(NPC, N)
        A[m * NPC:(m + 1) * NPC] = blk == 1

    return _host_labels(A)


# revision 6
# speedup vs baseline: 7.4789x; 1.0977x over previous
"""DBSCAN (eps=22, min_samples=5) on X[8192, 256] float32, distributed
across 8 TRN2 NeuronCores via Bass/Tile.

Math (mirrors the jax reference):
  d2[i,j] = ||x_i||^2 + ||x_j||^2 - 2 (X X^T)[i,j]
  adj     = d2 <= eps^2   <=>   G[i,j] >= thr_i + thr_j,
            thr = ||x||^2/2 - eps^2/4
  core_i  = rowsum(adj) >= min_samples
  comp    = min-index connected components of the core-core eps-graph
  labels  = component ids in scan order; border points attach to the
            min-index core neighbor; rest are noise (-1).

Device (the compute-bound part): core m owns rows [1024*m, 1024*(m+1)).
Each core computes its [1024, 8192] Gram block on the tensor engine
(bf16 inputs, fp32 PSUM) and binarizes each [128, 1024] PSUM tile into
0/1 uint8 adjacency bytes:
  - 3/4 of tiles: DVE scalar_tensor_tensor
        adj = (g - thr_i) is_ge thr_j    (thr_j from an f32 broadcast tile)
  - 1/4 of tiles: a K=2 bf16 hi/lo matmul folds -thr_j into PSUM, then
    the scalar engine evicts with Sign(g~ - thr_i) (bias per partition);
    decode on host is byte == 1 either way.
The loop runs column-block-outer so inputs for block b+1 and adjacency
outputs of block b stream on the three DMA queues (SP / Activation /
gpsimd) while the PE computes block b. No collectives; the PE Gram is
the critical path.

Host: degrees (popcount), connected components of the core-core graph
(packed-bit BFS in increasing index order, so each component's label is
its min core index — exactly the reference's propagation fixpoint),
border attachment, and scan-order cluster numbering. All exact integer
math on the device-computed adjacency.
"""

import numpy as np
import ml_dtypes

N = 8192
D = 256
NCORES = 8
NPC = N // NCORES          # 1024 rows per core
RCH = NPC // 128           # 8 row-chunks of 128 per core
NB = 4                     # column blocks
BW = N // NB               # 2048 columns per block
EPS2 = 484.0               # 22.0**2
MIN_SAMPLES = 5
BIG = N

_CACHE = {}


def _build_nc():
    import concourse.bass as bass
    import concourse.bacc as bacc
    import concourse.tile as tile
    import concourse.mybir as mybir

    f32 = mybir.dt.float32
    bf16 = mybir.dt.bfloat16
    u8 = mybir.dt.uint8
    Alu = mybir.AluOpType
    Act = mybir.ActivationFunctionType

    nc = bacc.Bacc("TRN2", target_bir_lowering=False, debug=False,
                   num_devices=NCORES)

    # ---- kernel I/O ----
    xt_d = nc.declare_dram_parameter("xt", [2, 128, N], bf16, isOutput=False)
    xo_d = nc.declare_dram_parameter("xtown", [2, 128, NPC], bf16,
                                     isOutput=False)
    cth_d = nc.declare_dram_parameter("cth", [128, N], f32, isOutput=False)
    cn_d = nc.declare_dram_parameter("cneg", [2, N], bf16, isOutput=False)
    rj_d = nc.declare_dram_parameter("rj", [128, RCH], f32, isOutput=False)
    rjn_d = nc.declare_dram_parameter("rjn", [128, RCH], f32, isOutput=False)

    adj_o = nc.declare_dram_parameter("adj", [RCH, NB, 128, BW], u8,
                                      isOutput=True)

    with tile.TileContext(nc) as tc:
        with (
            tc.tile_pool(name="inp", bufs=1) as inp,
            tc.tile_pool(name="adjp", bufs=1) as adjp,
            tc.tile_pool(name="ps", bufs=4, space=bass.MemorySpace.PSUM) as ps,
        ):
            # q0 (gpsimd): own rows + xt1 blocks; q1 (sync): xt0 blocks;
            # q10 (scalar): smalls + cth blocks.  Outputs rotate over all 3.
            xo0 = inp.tile([128, NPC], bf16, tag="xo0")
            nc.gpsimd.dma_start(xo0[:], xo_d[0])
            xo1 = inp.tile([128, NPC], bf16, tag="xo1")
            nc.gpsimd.dma_start(xo1[:], xo_d[1])
            rj = inp.tile([128, RCH], f32, tag="rj")
            nc.scalar.dma_start(rj[:], rj_d[:])
            rjn = inp.tile([128, RCH], f32, tag="rjn")
            nc.scalar.dma_start(rjn[:], rjn_d[:])
            cn = inp.tile([2, N], bf16, tag="cn")
            nc.scalar.dma_start(cn[:], cn_d[:])
            ones2 = inp.tile([2, 128], bf16, tag="ones2")
            nc.vector.memset(ones2[:], 1.0)

            xtg = [[None] * NB for _ in range(2)]
            cthg = [None] * NB
            adjt = [[adjp.tile([128, BW], u8, tag=f"adj{r}b{b}",
                               name=f"adj{r}b{b}")
                     for b in range(NB)] for r in range(RCH)]

            def load_block(b):
                sl = slice(b * BW, (b + 1) * BW)
                t0 = inp.tile([128, BW], bf16, tag=f"xt0g{b}",
                              name=f"xt0g{b}")
                nc.sync.dma_start(t0[:], xt_d[0][:, sl])
                xtg[0][b] = t0
                t1 = inp.tile([128, BW], bf16, tag=f"xt1g{b}",
                              name=f"xt1g{b}")
                nc.gpsimd.dma_start(t1[:], xt_d[1][:, sl])
                xtg[1][b] = t1
                tcth = inp.tile([128, BW], f32, tag=f"cthg{b}",
                                name=f"cthg{b}")
                nc.scalar.dma_start(tcth[:], cth_d[:, sl])
                cthg[b] = tcth

            load_block(0)
            out_eng = [nc.gpsimd, nc.sync, nc.scalar]

            for b in range(NB):
                if b + 1 < NB:
                    load_block(b + 1)
                for r in range(RCH):
                    l0 = xo0[:, r * 128:(r + 1) * 128]
                    l1 = xo1[:, r * 128:(r + 1) * 128]
                    for w in range(2):      # two [128, 1024] tiles per block
                        col0 = b * BW + w * 1024
                        act_tile = (w == 1 and r % 2 == 1)
                        g = ps.tile([128, 1024], f32, tag="g", name="g")
                        for hh in range(2):  # two 512-wide matmul groups
                            iw = w * 1024 + hh * 512
                            rsl = slice(iw, iw + 512)
                            osl = slice(hh * 512, hh * 512 + 512)
                            nc.tensor.matmul(g[:, osl], l0,
                                             xtg[0][b][:, rsl],
                                             start=True, stop=False)
                            if act_tile:
                                nc.tensor.matmul(g[:, osl], l1,
                                                 xtg[1][b][:, rsl],
                                                 start=False, stop=False)
                                csl = slice(col0 + hh * 512,
                                            col0 + hh * 512 + 512)
                                nc.tensor.matmul(g[:, osl], ones2[:],
                                                 cn[:, csl],
                                                 start=False, stop=True)
                            else:
                                nc.tensor.matmul(g[:, osl], l1,
                                                 xtg[1][b][:, rsl],
                                                 start=False, stop=True)
                        osl = slice(w * 1024, w * 1024 + 1024)
                        if act_tile:
                            # adj = Sign(g~ - thr_i): +1 -> byte 1
                            nc.scalar.activation(
                                adjt[r][b][:, osl], g[:], Act.Sign,
                                bias=rjn[:, r:r + 1], scale=1.0)
                        else:
                            # adj = (g - thr_i) >= thr_j
                            nc.vector.scalar_tensor_tensor(
                                out=adjt[r][b][:, osl], in0=g[:],
                                scalar=rj[:, r:r + 1],
                                in1=cthg[b][:, w * 1024:w * 1024 + 1024],
                                op0=Alu.subtract, op1=Alu.is_ge)
                    out_eng[r % 3].dma_start(adj_o[r][b], adjt[r][b][:])

    nc.compile()
    return nc


def _prepare_inputs(X):
    X = np.ascontiguousarray(X, dtype=np.float32)
    sq = np.sum(X * X, axis=1, dtype=np.float32)          # [N]
    # adj  <=>  G >= thr_i + thr_j
    thr = sq * np.float32(0.5) - np.float32(EPS2 / 4.0)   # [N]

    xt_bf = X.T.astype(ml_dtypes.bfloat16)                # [256, 8192]
    xt = np.ascontiguousarray(xt_bf.reshape(2, 128, N))

    cth = np.ascontiguousarray(np.broadcast_to(thr, (128, N)))

    cneg_f = (-thr).astype(np.float32)
    hi = cneg_f.astype(ml_dtypes.bfloat16)
    lo = (cneg_f - hi.astype(np.float32)).astype(ml_dtypes.bfloat16)
    cneg = np.ascontiguousarray(np.stack([hi, lo], axis=0))  # [2, 8192]

    in_maps = []
    for m in range(NCORES):
        rows = np.arange(m * NPC, (m + 1) * NPC)
        # chunk r holds local rows r*128 .. r*128+127 -> rj[q, r]
        rj = np.ascontiguousarray(thr[rows].reshape(RCH, 128).T)
        in_maps.append({
            "xt": xt,
            "xtown": np.ascontiguousarray(xt_bf[:, rows].reshape(2, 128, NPC)),
            "cth": cth,
            "cneg": cneg,
            "rj": rj,
            "rjn": np.ascontiguousarray(-rj),
        })
    return in_maps


def _host_finish(deg, bord, comp):
    """Exact numpy port of the reference's label-numbering tail."""
    idx = np.arange(N, dtype=np.int64)
    core = deg >= MIN_SAMPLES
    is_rep = core & (comp == idx)
    cid = np.cumsum(is_rep.astype(np.int64)) - 1
    comp_safe = np.minimum(comp, N - 1)
    core_label = np.where(core, cid[comp_safe], -1)
    first_core_nb = bord
    has_nb = first_core_nb < N
    nb_safe = np.minimum(first_core_nb, N - 1)
    border_label = np.where(has_nb, core_label[nb_safe], -1)
    return np.where(core, core_label, border_label).astype(np.int64)


def _host_labels(A):
    """Exact DBSCAN labeling from the 0/1 uint8 adjacency matrix."""
    deg = A.sum(axis=1, dtype=np.int64)
    core = deg >= MIN_SAMPLES
    idx = np.arange(N, dtype=np.int64)

    # core-core adjacency as packed bits for fast BFS
    core_u8 = core.astype(np.uint8)
    bits = np.packbits(A * core_u8[None, :], axis=1)      # [N, N//8]

    comp = np.full(N, BIG, dtype=np.int64)
    visited = np.zeros(N, dtype=bool)
    for i in np.nonzero(core)[0]:
        if visited[i]:
            continue
        member = np.zeros(N // 8, dtype=np.uint8)
        member[i >> 3] = 0x80 >> (i & 7)
        frontier = np.array([i], dtype=np.int64)
        while frontier.size:
            reach = np.bitwise_or.reduce(bits[frontier], axis=0)
            new = reach & ~member
            member |= new
            frontier = np.nonzero(np.unpackbits(new))[0]
        rows = np.nonzero(np.unpackbits(member))[0]
        comp[rows] = i          # scan order => i is the min index
        visited[rows] = True

    # border points: min-index core neighbor
    bord = np.full(N, BIG, dtype=np.int64)
    nonc = np.nonzero(~core)[0]
    if nonc.size:
        ub = np.unpackbits(bits[nonc], axis=1)[:, :N]
        has = ub.any(axis=1)
        bord[nonc[has]] = ub[has].argmax(axis=1)

    return _host_finish(deg, bord, comp)


def _run_device(in_maps):
    from concourse import bass_utils
    if "nc" not in _CACHE:
        _CACHE["nc"] = _build_nc()
    res = bass_utils.run_bass_kernel_spmd(
        _CACHE["nc"], in_maps, list(range(NCORES)))
    return res.results


def kernel(X):
    in_maps = _prepare_inputs(X)
    results = _run_device(in_maps)

    A = np.empty((N, N), dtype=np.uint8)
    for m in range(NCORES):
        blk = np.asarray(results[m]["adj"])       # [RCH, NB, 128, BW]
        blk = blk.transpose(0, 2, 1, 3).reshape(NPC, N)
        A[m * NPC:(m + 1) * NPC] = blk == 1

    return _host_labels(A)


# revision 25
# speedup vs baseline: 8.9842x; 1.2013x over previous
"""DBSCAN (eps=22, min_samples=5) on X[8192, 256] float32, distributed
across 8 TRN2 NeuronCores via Bass/Tile.

Math (mirrors the jax reference):
  d2[i,j] = ||x_i||^2 + ||x_j||^2 - 2 (X X^T)[i,j]
  adj     = d2 <= eps^2   <=>   G[i,j] >= thr_i + thr_j,
            thr = ||x||^2/2 - eps^2/4
  core_i  = rowsum(adj) >= min_samples
  comp    = min-index connected components of the core-core eps-graph
  labels  = component ids in scan order; border points attach to the
            min-index core neighbor; rest are noise (-1).

Device (the compute-bound part): core m owns rows [1024*m, 1024*(m+1)).
Each core computes its [1024, 8192] Gram block on the tensor engine
(bf16 inputs, fp32 PSUM) and evicts each [128, 1024] PSUM tile as
bf16(g - thr_i) on two lanes that keep pace with the PE: half through
the DVE (tensor_scalar subtract of the per-partition thr_i), half
through the scalar engine (Identity activation with per-partition
bias). The loop runs column-block-outer so inputs for block b+1 and
outputs of block b stream on the DMA queues while the PE computes
block b. No collectives; the PE Gram is the critical path.

Host: the threshold compare bf16(g - thr_i) >= thr_j (the bf16
rounding is far below the adjacency decision scale), then degrees
(popcount), connected components of the core-core graph (packed-bit
BFS in increasing index order, so each component's label is its min
core index — exactly the reference's propagation fixpoint), border
attachment, and scan-order cluster numbering.
"""

import numpy as np
import ml_dtypes

N = 8192
D = 256
NCORES = 8
NPC = N // NCORES          # 1024 rows per core
RCH = NPC // 128           # 8 row-chunks of 128 per core
NB = 4                     # column blocks
BW = N // NB               # 2048 columns per block
EPS2 = 484.0               # 22.0**2
MIN_SAMPLES = 5
BIG = N

_CACHE = {}


def _build_nc():
    import concourse.bass as bass
    import concourse.bacc as bacc
    import concourse.tile as tile
    import concourse.mybir as mybir

    f32 = mybir.dt.float32
    bf16 = mybir.dt.bfloat16
    u8 = mybir.dt.uint8
    Alu = mybir.AluOpType
    Act = mybir.ActivationFunctionType

    nc = bacc.Bacc("TRN2", target_bir_lowering=False, debug=False,
                   num_devices=NCORES)

    # ---- kernel I/O ----
    xt_d = nc.declare_dram_parameter("xt", [2, 128, N], bf16, isOutput=False)
    xo_d = nc.declare_dram_parameter("xtown", [2, 128, NPC], bf16,
                                     isOutput=False)
    rj_d = nc.declare_dram_parameter("rj", [128, RCH], f32, isOutput=False)
    rjn_d = nc.declare_dram_parameter("rjn", [128, RCH], f32, isOutput=False)

    adj_o = nc.declare_dram_parameter("adj", [RCH, NB, 128, BW], bf16,
                                      isOutput=True)

    with tile.TileContext(nc) as tc:
        with (
            tc.tile_pool(name="inp", bufs=1) as inp,
            tc.tile_pool(name="adjp", bufs=1) as adjp,
            tc.tile_pool(name="ps", bufs=4, space=bass.MemorySpace.PSUM) as ps,
        ):
            # inputs ride the two hardware DGE queues: q1 (sync) gets the
            # xt0 blocks, q10 (scalar) the thresholds, own rows, xt1 blocks
            rj = inp.tile([128, RCH], f32, tag="rj")
            nc.scalar.dma_start(rj[:], rj_d[:])
            rjn = inp.tile([128, RCH], f32, tag="rjn")
            nc.scalar.dma_start(rjn[:], rjn_d[:])
            xo0 = inp.tile([128, NPC], bf16, tag="xo0")
            nc.scalar.dma_start(xo0[:], xo_d[0])
            xo1 = inp.tile([128, NPC], bf16, tag="xo1")
            nc.scalar.dma_start(xo1[:], xo_d[1])

            xtg = [[None] * NB for _ in range(2)]
            adjt = [[adjp.tile([128, BW], bf16, tag=f"adj{r}b{b}",
                               name=f"adj{r}b{b}")
                     for b in range(NB)] for r in range(RCH)]

            def load_block(b):
                sl = slice(b * BW, (b + 1) * BW)
                t0 = inp.tile([128, BW], bf16, tag=f"xt0g{b}",
                              name=f"xt0g{b}")
                nc.sync.dma_start(t0[:], xt_d[0][:, sl])
                xtg[0][b] = t0
                t1 = inp.tile([128, BW], bf16, tag=f"xt1g{b}",
                              name=f"xt1g{b}")
                nc.scalar.dma_start(t1[:], xt_d[1][:, sl])
                xtg[1][b] = t1

            load_block(0)
            out_eng = [nc.gpsimd, nc.sync, nc.scalar]

            for b in range(NB):
                if b + 1 < NB:
                    load_block(b + 1)
                for r in range(RCH):
                    l0 = xo0[:, r * 128:(r + 1) * 128]
                    l1 = xo1[:, r * 128:(r + 1) * 128]
                    for w in range(2):      # two [128, 1024] tiles per block
                        g = ps.tile([128, 1024], f32, tag="g", name="g")
                        for hh in range(2):  # two 512-wide matmul groups
                            iw = w * 1024 + hh * 512
                            rsl = slice(iw, iw + 512)
                            osl = slice(hh * 512, hh * 512 + 512)
                            nc.tensor.matmul(g[:, osl], l0,
                                             xtg[0][b][:, rsl],
                                             start=True, stop=False)
                            nc.tensor.matmul(g[:, osl], l1,
                                             xtg[1][b][:, rsl],
                                             start=False, stop=True)
                        # evict bf16(g - thr_i); host compares vs thr_j
                        osl = slice(w * 1024, w * 1024 + 1024)
                        if (r + w) % 2 == 0:
                            nc.vector.tensor_scalar(
                                out=adjt[r][b][:, osl], in0=g[:],
                                scalar1=rj[:, r:r + 1], scalar2=None,
                                op0=Alu.subtract)
                        else:
                            nc.scalar.activation(
                                adjt[r][b][:, osl], g[:], Act.Identity,
                                bias=rjn[:, r:r + 1], scale=1.0)
                    out_eng[r % 3].dma_start(adj_o[r][b], adjt[r][b][:])

    nc.compile()
    return nc


def _prepare_inputs(X):
    X = np.ascontiguousarray(X, dtype=np.float32)
    sq = np.sum(X * X, axis=1, dtype=np.float32)          # [N]
    # adj  <=>  G >= thr_i + thr_j
    thr = sq * np.float32(0.5) - np.float32(EPS2 / 4.0)   # [N]

    xt_bf = X.T.astype(ml_dtypes.bfloat16)                # [256, 8192]
    xt = np.ascontiguousarray(xt_bf.reshape(2, 128, N))

    in_maps = []
    for m in range(NCORES):
        rows = np.arange(m * NPC, (m + 1) * NPC)
        # chunk r holds local rows r*128 .. r*128+127 -> rj[q, r]
        rj = np.ascontiguousarray(thr[rows].reshape(RCH, 128).T)
        in_maps.append({
            "xt": xt,
            "xtown": np.ascontiguousarray(xt_bf[:, rows].reshape(2, 128, NPC)),
            "rj": rj,
            "rjn": np.ascontiguousarray(-rj),
        })
    return in_maps


def _host_finish(deg, bord, comp):
    """Exact numpy port of the reference's label-numbering tail."""
    idx = np.arange(N, dtype=np.int64)
    core = deg >= MIN_SAMPLES
    is_rep = core & (comp == idx)
    cid = np.cumsum(is_rep.astype(np.int64)) - 1
    comp_safe = np.minimum(comp, N - 1)
    core_label = np.where(core, cid[comp_safe], -1)
    first_core_nb = bord
    has_nb = first_core_nb < N
    nb_safe = np.minimum(first_core_nb, N - 1)
    border_label = np.where(has_nb, core_label[nb_safe], -1)
    return np.where(core, core_label, border_label).astype(np.int64)


def _host_labels(A):
    """Exact DBSCAN labeling from the 0/1 uint8 adjacency matrix."""
    deg = A.sum(axis=1, dtype=np.int64)
    core = deg >= MIN_SAMPLES
    idx = np.arange(N, dtype=np.int64)

    # core-core adjacency as packed bits for fast BFS
    core_u8 = core.astype(np.uint8)
    bits = np.packbits(A * core_u8[None, :], axis=1)      # [N, N//8]

    comp = np.full(N, BIG, dtype=np.int64)
    visited = np.zeros(N, dtype=bool)
    for i in np.nonzero(core)[0]:
        if visited[i]:
            continue
        member = np.zeros(N // 8, dtype=np.uint8)
        member[i >> 3] = 0x80 >> (i & 7)
        frontier = np.array([i], dtype=np.int64)
        while frontier.size:
            reach = np.bitwise_or.reduce(bits[frontier], axis=0)
            new = reach & ~member
            member |= new
            frontier = np.nonzero(np.unpackbits(new))[0]
        rows = np.nonzero(np.unpackbits(member))[0]
        comp[rows] = i          # scan order => i is the min index
        visited[rows] = True

    # border points: min-index core neighbor
    bord = np.full(N, BIG, dtype=np.int64)
    nonc = np.nonzero(~core)[0]
    if nonc.size:
        ub = np.unpackbits(bits[nonc], axis=1)[:, :N]
        has = ub.any(axis=1)
        bord[nonc[has]] = ub[has].argmax(axis=1)

    return _host_finish(deg, bord, comp)


def _run_device(in_maps):
    from concourse import bass_utils
    if "nc" not in _CACHE:
        _CACHE["nc"] = _build_nc()
    res = bass_utils.run_bass_kernel_spmd(
        _CACHE["nc"], in_maps, list(range(NCORES)))
    return res.results


def kernel(X):
    in_maps = _prepare_inputs(X)
    results = _run_device(in_maps)

    X = np.ascontiguousarray(X, dtype=np.float32)
    sq = np.sum(X * X, axis=1, dtype=np.float32)
    thr = sq * np.float32(0.5) - np.float32(EPS2 / 4.0)

    A = np.empty((N, N), dtype=np.uint8)
    for m in range(NCORES):
        blk = np.asarray(results[m]["adj"])       # [RCH, NB, 128, BW] bf16
        blk = blk.transpose(0, 2, 1, 3).reshape(NPC, N)
        # g - thr_i (bf16) >= thr_j
        A[m * NPC:(m + 1) * NPC] = blk.astype(np.float32) >= thr[None, :]

    return _host_labels(A)


# revision 26
# speedup vs baseline: 13.7330x; 1.5286x over previous
"""DBSCAN (eps=22, min_samples=5) on X[8192, 256] float32, distributed
across 8 TRN2 NeuronCores via Bass/Tile.

Math (mirrors the jax reference):
  d2[i,j] = ||x_i||^2 + ||x_j||^2 - 2 (X X^T)[i,j]
  adj     = d2 <= eps^2   <=>   G[i,j] >= thr_i + thr_j,
            thr = ||x||^2/2 - eps^2/4
  core_i  = rowsum(adj) >= min_samples
  comp    = min-index connected components of the core-core eps-graph
  labels  = component ids in scan order; border points attach to the
            min-index core neighbor; rest are noise (-1).

Device (the compute-bound part): the Gram matrix is symmetric, so each
512-row half-chunk only computes the circulant column range
[512*h, 512*h + 4608) mod 8192 (9 of 16 half-blocks; for any pair at
least one direction is covered, host mirrors the rest). Core m owns
rows [1024*m, 1024*(m+1)) and receives xt pre-rolled by 1024*m
columns, which keeps the program identical across cores (SPMD). The PE
computes [128, 512] Gram tiles (bf16 inputs, fp32 PSUM, 144 matmuls =
56% of the naive count) and two eviction lanes that keep pace with it
write bf16(g - thr_i) to SBUF: the DVE (tensor_scalar subtract of the
per-partition thr_i) and the scalar engine (Identity activation with
per-partition bias). Inputs (3 MB) and the per-column-slice outputs
stream on the DMA queues while the PE computes. No collectives.

Host: the threshold compare bf16(g - thr_i) >= thr_j (the bf16
rounding is far below the adjacency decision scale), symmetric closure
A |= A.T, then degrees, connected components of the core-core graph
(packed-bit BFS in increasing index order, so each component's label
is its min core index — exactly the reference's propagation fixpoint),
border attachment, and scan-order cluster numbering.
"""

import numpy as np
import ml_dtypes

N = 8192
D = 256
NCORES = 8
NPC = N // NCORES          # 1024 rows per core
RCH = NPC // 128           # 8 row-chunks of 128 per core
NS = 10                    # 512-col sub-blocks of the per-core xt slice
CW = NS * 512              # 5120 columns staged per core
KS = 9                     # sub-blocks covered per 512-row half-chunk
EPS2 = 484.0               # 22.0**2
MIN_SAMPLES = 5
BIG = N

_CACHE = {}


def _build_nc():
    import concourse.bass as bass
    import concourse.bacc as bacc
    import concourse.tile as tile
    import concourse.mybir as mybir

    f32 = mybir.dt.float32
    bf16 = mybir.dt.bfloat16
    Alu = mybir.AluOpType
    Act = mybir.ActivationFunctionType

    nc = bacc.Bacc("TRN2", target_bir_lowering=False, debug=False,
                   num_devices=NCORES)

    # ---- kernel I/O ----
    xt_d = nc.declare_dram_parameter("xt", [2, 128, CW], bf16, isOutput=False)
    xo_d = nc.declare_dram_parameter("xtown", [2, 128, NPC], bf16,
                                     isOutput=False)
    rj_d = nc.declare_dram_parameter("rj", [128, RCH], f32, isOutput=False)
    rjn_d = nc.declare_dram_parameter("rjn", [128, RCH], f32, isOutput=False)

    # per sub-block s: row-chunks t (8 segments of 512 cols, some unused)
    adj_o = nc.declare_dram_parameter("adj", [NS, 128, RCH * 512], bf16,
                                      isOutput=True)

    with tile.TileContext(nc) as tc:
        with (
            tc.tile_pool(name="inp", bufs=1) as inp,
            tc.tile_pool(name="adjp", bufs=1) as adjp,
            tc.tile_pool(name="ps", bufs=8, space=bass.MemorySpace.PSUM) as ps,
        ):
            # inputs ride the two hardware DGE queues
            rj = inp.tile([128, RCH], f32, tag="rj")
            nc.scalar.dma_start(rj[:], rj_d[:])
            rjn = inp.tile([128, RCH], f32, tag="rjn")
            nc.scalar.dma_start(rjn[:], rjn_d[:])
            xo0 = inp.tile([128, NPC], bf16, tag="xo0")
            nc.sync.dma_start(xo0[:], xo_d[0])
            xo1 = inp.tile([128, NPC], bf16, tag="xo1")
            nc.scalar.dma_start(xo1[:], xo_d[1])

            xtg = [[None] * NS for _ in range(2)]

            def load_sub(s):
                sl = slice(s * 512, (s + 1) * 512)
                t0 = inp.tile([128, 512], bf16, tag=f"xt0g{s}",
                              name=f"xt0g{s}")
                nc.sync.dma_start(t0[:], xt_d[0][:, sl])
                xtg[0][s] = t0
                t1 = inp.tile([128, 512], bf16, tag=f"xt1g{s}",
                              name=f"xt1g{s}")
                nc.scalar.dma_start(t1[:], xt_d[1][:, sl])
                xtg[1][s] = t1

            load_sub(0)
            load_sub(1)
            adjt = [adjp.tile([128, RCH * 512], bf16, tag=f"adj{s}",
                              name=f"adj{s}") for s in range(NS)]
            out_eng = [nc.gpsimd, nc.sync, nc.scalar]
            lane = 0

            for s in range(NS):
                if s + 2 < NS:
                    load_sub(s + 2)
                # chunk A (t=0..3) covers s<=KS-1; chunk B (t=4..7) s>=NS-KS
                tlist = []
                if s <= KS - 1:
                    tlist += [0, 1, 2, 3]
                if s >= NS - KS:
                    tlist += [4, 5, 6, 7]
                for t in tlist:
                    l0 = xo0[:, t * 128:(t + 1) * 128]
                    l1 = xo1[:, t * 128:(t + 1) * 128]
                    g = ps.tile([128, 512], f32, tag="g", name="g")
                    nc.tensor.matmul(g[:], l0, xtg[0][s][:],
                                     start=True, stop=False)
                    nc.tensor.matmul(g[:], l1, xtg[1][s][:],
                                     start=False, stop=True)
                    # evict bf16(g - thr_i); host compares vs thr_j
                    osl = slice(t * 512, t * 512 + 512)
                    if lane == 0:
                        nc.vector.tensor_scalar(
                            out=adjt[s][:, osl], in0=g[:],
                            scalar1=rj[:, t:t + 1], scalar2=None,
                            op0=Alu.subtract)
                    else:
                        nc.scalar.activation(
                            adjt[s][:, osl], g[:], Act.Identity,
                            bias=rjn[:, t:t + 1], scale=1.0)
                    lane ^= 1
                lo = tlist[0] * 512
                hi = (tlist[-1] + 1) * 512
                out_eng[s % 3].dma_start(adj_o[s][:, lo:hi],
                                         adjt[s][:, lo:hi])

    nc.compile()
    return nc


def _prepare_inputs(X):
    X = np.ascontiguousarray(X, dtype=np.float32)
    sq = np.sum(X * X, axis=1, dtype=np.float32)          # [N]
    # adj  <=>  G >= thr_i + thr_j
    thr = sq * np.float32(0.5) - np.float32(EPS2 / 4.0)   # [N]

    xt_bf = X.T.astype(ml_dtypes.bfloat16)                # [256, 8192]

    in_maps = []
    for m in range(NCORES):
        rows = np.arange(m * NPC, (m + 1) * NPC)
        cols = (m * NPC + np.arange(CW)) % N
        rj = np.ascontiguousarray(thr[rows].reshape(RCH, 128).T)
        in_maps.append({
            "xt": np.ascontiguousarray(
                xt_bf[:, cols].reshape(2, 128, CW)),
            "xtown": np.ascontiguousarray(
                xt_bf[:, rows].reshape(2, 128, NPC)),
            "rj": rj,
            "rjn": np.ascontiguousarray(-rj),
        })
    return in_maps


def _host_finish(deg, bord, comp):
    """Exact numpy port of the reference's label-numbering tail."""
    idx = np.arange(N, dtype=np.int64)
    core = deg >= MIN_SAMPLES
    is_rep = core & (comp == idx)
    cid = np.cumsum(is_rep.astype(np.int64)) - 1
    comp_safe = np.minimum(comp, N - 1)
    core_label = np.where(core, cid[comp_safe], -1)
    first_core_nb = bord
    has_nb = first_core_nb < N
    nb_safe = np.minimum(first_core_nb, N - 1)
    border_label = np.where(has_nb, core_label[nb_safe], -1)
    return np.where(core, core_label, border_label).astype(np.int64)


def _host_labels(A):
    """Exact DBSCAN labeling from the 0/1 uint8 adjacency matrix."""
    deg = A.sum(axis=1, dtype=np.int64)
    core = deg >= MIN_SAMPLES
    idx = np.arange(N, dtype=np.int64)

    # core-core adjacency as packed bits for fast BFS
    core_u8 = core.astype(np.uint8)
    bits = np.packbits(A * core_u8[None, :], axis=1)      # [N, N//8]

    comp = np.full(N, BIG, dtype=np.int64)
    visited = np.zeros(N, dtype=bool)
    for i in np.nonzero(core)[0]:
        if visited[i]:
            continue
        member = np.zeros(N // 8, dtype=np.uint8)
        member[i >> 3] = 0x80 >> (i & 7)
        frontier = np.array([i], dtype=np.int64)
        while frontier.size:
            reach = np.bitwise_or.reduce(bits[frontier], axis=0)
            new = reach & ~member
            member |= new
            frontier = np.nonzero(np.unpackbits(new))[0]
        rows = np.nonzero(np.unpackbits(member))[0]
        comp[rows] = i          # scan order => i is the min index
        visited[rows] = True

    # border points: min-index core neighbor
    bord = np.full(N, BIG, dtype=np.int64)
    nonc = np.nonzero(~core)[0]
    if nonc.size:
        ub = np.unpackbits(bits[nonc], axis=1)[:, :N]
        has = ub.any(axis=1)
        bord[nonc[has]] = ub[has].argmax(axis=1)

    return _host_finish(deg, bord, comp)


def _run_device(in_maps):
    from concourse import bass_utils
    if "nc" not in _CACHE:
        _CACHE["nc"] = _build_nc()
    res = bass_utils.run_bass_kernel_spmd(
        _CACHE["nc"], in_maps, list(range(NCORES)))
    return res.results


def kernel(X):
    in_maps = _prepare_inputs(X)
    results = _run_device(in_maps)

    X = np.ascontiguousarray(X, dtype=np.float32)
    sq = np.sum(X * X, axis=1, dtype=np.float32)
    thr = sq * np.float32(0.5) - np.float32(EPS2 / 4.0)

    A = np.zeros((N, N), dtype=np.uint8)
    for m in range(NCORES):
        blk = np.asarray(results[m]["adj"])   # [NS, 128, RCH*512] bf16
        cols = (m * NPC + np.arange(CW)) % N
        for s in range(NS):
            csl = cols[s * 512:(s + 1) * 512]
            tlist = (list(range(4)) if s <= KS - 1 else []) + \
                    (list(range(4, 8)) if s >= NS - KS else [])
            for t in tlist:
                rows = slice(m * NPC + t * 128, m * NPC + (t + 1) * 128)
                vals = blk[s, :, t * 512:(t + 1) * 512].astype(np.float32)
                A[rows, csl] = vals >= thr[csl][None, :]

    A |= A.T
    return _host_labels(A)
